# revision 1
# baseline (speedup 1.0000x reference)
"""Class-align loss (segment_reduce) Trainium2 kernel.

Full inputs: f_source [4,256,128,128] f32, f_convert [4,256,128,128] f32,
seg [4,128,128] int32 (values in [0,19)). Output: scalar f32 triplet loss.

Strategy (data-parallel over batch*h-half, 8 shards):
  - Each core processes a [256, 8192] shard of each feature tensor
    (1 batch x 64 h-rows x 128 w). Staging DMAs cast fp32 -> bf16 in
    flight (SWDGE); accumulation stays fp32 in PSUM.
  - Pixels are processed in batches of 4 groups x 128 pixels: PE
    transposes eight [128c,128p] bf16 blocks into one full-bank PSUM
    tile, one DVE copy moves it to SBUF; ACT computes per-pixel
    sum-of-squares (Square + accum) per group; one batched sqrt (ACT)
    + reciprocal (DVE) per batch gives r = 1/||x||.
  - Per-pixel normalization is folded into the one-hot class weights
    w[p,k] = (seg[p]==k) * r[p]; PE accumulates transposed class sums
    accT[c_half,k] += xT_half.T @ w (data is the 128-col stationary
    operand -> fast weight load; w is the 19-col moving operand).
  - Each core writes its partial [2,2,128,19] sums; the host sums the
    8 partials and computes the tiny (19-class) normalize +
    triplet-loss epilogue in float64.

The walrus build used here encodes at most ONE sync wait per
instruction. Everything below is arranged so no instruction ever needs
two: staging tiles are dedicated (wait-free DMAs), absorber transposes
take the staging-DMA waits on PE, sync=False ordering edges keep the PE
stream near program order so PSUM-slot WAR waits are subsumed by the
vector clock, and the kernel-tail drain is split across sequencer NOPs.
"""

import sys

import numpy as np

if "/opt/trn_rl_repo" not in sys.path:
    sys.path.insert(0, "/opt/trn_rl_repo")

import concourse.bass as bass
import concourse.mybir as mybir
import concourse.tile as tile
from concourse.bass_utils import run_bass_kernel_spmd
from concourse.tile import add_dep_helper
from concourse.vector_clock import ScopedClock


def _split_drain_and_barrier(self, tick_clock, wait_clock):
    """Tile's kernel-tail drain carries one wait per semaphore the kernel
    ever used; split the excess onto dedicated sequencer NOPs (the 1-wait
    walrus encoding limit)."""
    nc = self.nc
    drain_inst = nc.sync.drain()
    wait_clock.add_sem_waits(
        drain_inst.ins, ScopedClock({None: tick_clock.global_clock})
    )
    si = drain_inst.ins.sync_info
    if si is not None and len(si.on_wait) > 1:
        waits = list(si.on_wait)
        upds = list(si.on_update)
        drain_inst.ins.sync_info = mybir.SyncInfo(
            on_wait=waits[:1], on_update=upds)
        for k in range(1, len(waits)):
            nop = nc.sync.nop(nofuse=True, hint=f"drain_wait_{k}")
            nop.ins.sync_info = mybir.SyncInfo(
                on_wait=[waits[k]], on_update=[])
    nc.all_engine_barrier()
    assert self.sems is not None
    popped = nc._tile_sem_poison_stack.pop()
    assert popped is self._sem_poison
    nc.clear_and_free_semaphores(list(self.sems.allocated().values()))
    nc.all_engine_barrier()


tile.TileContext._drain_and_barrier = _split_drain_and_barrier

# Problem constants (hardcoded; kernel.py must be self-contained).
B, C, H, W = 4, 256, 128, 128
N_CLASS = 19
N_CORES = 8
EPS_NORM = 1e-12
EPS_TRIP = 1e-6
MARGIN = 0.2

P = 128                      # SBUF partitions / pixel-group size
NPIX = B * H * W // N_CORES  # 8192 pixels per core
NG = NPIX // P               # 64 pixel groups per core
GPB = 4                      # pixel groups per batch (one PSUM bank)
NB = NG // GPB               # 16 batches per tensor
CHUNK_PIX = 4096             # pixels staged per DMA
NCHUNK = NPIX // CHUNK_PIX   # 4
BPC = CHUNK_PIX // (P * GPB) # 4 batches per chunk

_NC_CACHE = {}


def build_nc():
    f32 = mybir.dt.float32
    bf16 = mybir.dt.bfloat16
    i32 = mybir.dt.int32
    nc = bass.Bass()

    fs_dram = nc.declare_dram_parameter("f_source", [C, NPIX], f32, isOutput=False)
    aux_dram = nc.declare_dram_parameter("aux", [P, P + N_CLASS], f32,
                                         isOutput=False)
    fc_dram = nc.declare_dram_parameter("f_convert", [C, NPIX], f32, isOutput=False)
    seg_dram = nc.declare_dram_parameter("seg", [NPIX], i32, isOutput=False)
    out_dram = nc.declare_dram_parameter("out", [2, 2, P, N_CLASS], f32,
                                         isOutput=True)

    with tile.TileContext(nc) as tc:
        with (
            tc.tile_pool(name="const", bufs=1) as const_pool,
            tc.tile_pool(name="stage", bufs=1) as stage_pool,
            tc.tile_pool(name="work", bufs=4) as work_pool,
            tc.tile_pool(name="wpool", bufs=256) as w_pool,
            tc.tile_pool(name="psum_t", bufs=3, space="PSUM") as psum_t_pool,
            tc.tile_pool(name="psum_abs", bufs=1, space="PSUM") as psum_abs_pool,
            tc.tile_pool(name="psum_acc", bufs=1, space="PSUM") as psum_acc_pool,
        ):
            # identity + iota row arrive via DMA (the "aux" input): building
            # them with gpsimd would add the Pool semaphore to every
            # wait-budget discussion.
            aux_sb = const_pool.tile([P, P + N_CLASS], f32, tag="aux")
            nc.gpsimd.dma_start(out=aux_sb[:], in_=aux_dram[:])
            iota19 = aux_sb[:, P:P + N_CLASS]
            ident_bf = const_pool.tile([P, P], bf16, tag="ident_bf")
            nc.vector.tensor_copy(ident_bf[:], aux_sb[:, 0:P])
            identity = ident_bf[:]

            # seg wanted as [pixel-within-group (partition), group (free)].
            # A strided gather DMA would cost 8192 descriptors (~48us of Q7
            # descriptor generation), so load contiguously and PE-transpose.
            seg_i = const_pool.tile([NG, P], i32, tag="seg_i")
            nc.gpsimd.dma_start(
                out=seg_i[:], in_=seg_dram[:].rearrange("(g p) -> g p", p=P))
            seg_f = const_pool.tile([NG, P], f32, tag="seg_f")
            nc.vector.tensor_copy(seg_f[:], seg_i[:])
            ident_sm = const_pool.tile([NG, NG], f32, tag="ident_sm")
            nc.vector.tensor_copy(ident_sm[:], aux_sb[:NG, :NG])
            seg_ps = psum_t_pool.tile([P, NG], f32, tag="pt", name="seg_ps",
                                      padded_shape=[P, 512])
            nc.tensor.transpose(seg_ps[:], seg_f[:], ident_sm[:])
            seg_sb = const_pool.tile([P, NG], f32, tag="seg")
            nc.vector.tensor_copy(seg_sb[:], seg_ps[:])

            # Dummy DVE + gpsimd reads of iota19: pre-sync both engines
            # against the aux DMA so w-generation ops carry a single wait.
            iota_warm = const_pool.tile([P, N_CLASS], f32, tag="iota_warm")
            nc.vector.tensor_copy(iota_warm[:], iota19)


            # Warm-up transpose: pre-syncs PE against ident_bf (DVE).
            warm = psum_t_pool.tile([P, P], bf16, tag="pt", name="warm",
                                    padded_shape=[P, 1024])
            nc.tensor.transpose(warm[:, 0:P], identity, identity)

            # Transposed fp32 accumulators: accT[tensor][half] = [c_half, k].
            accs = {
                (t, h): psum_acc_pool.tile([P, N_CLASS], f32,
                                           tag=f"acc_{t}{h}", name=f"acc_{t}{h}")
                for t in ("s", "c") for h in (0, 1)
            }
            drams = {"s": fs_dram, "c": fc_dram}

            # Dedicated bank for the DMA-wait absorber transposes (never
            # read; lo/hi slices are byte-disjoint).
            absorb = psum_abs_pool.tile([P, 8 * P], bf16, tag="absorb",
                                        name="absorb", padded_shape=[P, 1024])

            mm_all = []

            def order_after_mm(inst, back=24):
                if len(mm_all) >= back:
                    add_dep_helper(inst.ins, mm_all[-back].ins, sync=False,
                                   reason="keep PE stream near program order")

            # Progressive chunk sizes: a small first chunk reaches SBUF
            # after ~6us of Q7 descriptor generation instead of ~24us, so
            # compute ramps while the big chunks' descriptors generate.
            chunks = [(0, 1024), (1024, 1024), (2048, 2048), (4096, 4096)]
            cbase = 0
            for ci, (pix0, cpix) in enumerate(chunks):
                for t in ("s", "c"):
                    # Dedicated staging tiles per (chunk, tensor, half): the
                    # DMAs carry zero waits. SWDGE casts fp32->bf16 in flight.
                    lo = stage_pool.tile([P, cpix], bf16,
                                         tag=f"{t}_lo_{ci}", name=f"{t}_lo_{ci}")
                    hi = stage_pool.tile([P, cpix], bf16,
                                         tag=f"{t}_hi_{ci}", name=f"{t}_hi_{ci}")
                    d1 = nc.gpsimd.dma_start(
                        out=lo[:], in_=drams[t][0:P, pix0:pix0 + cpix])
                    d2 = nc.gpsimd.dma_start(
                        out=hi[:], in_=drams[t][P:C, pix0:pix0 + cpix])
                    if ci >= 1:
                        # Keep later chunks' Q7 descriptor generation from
                        # bunching at kernel start: it would occupy the Q7
                        # for ~40us straight and delay everything behind it.
                        order_after_mm(d1, back=48)
                        order_after_mm(d2, back=48)
                    ab1 = nc.tensor.transpose(absorb[:, 0:P], lo[:, 0:P],
                                              identity)
                    ab2 = nc.tensor.transpose(absorb[:, P:2 * P], hi[:, 0:P],
                                              identity)
                    order_after_mm(ab1)
                    order_after_mm(ab2)
                    for bi in range(cpix // (P * GPB)):
                        batch = cbase + bi
                        # One full PSUM bank holds 4 groups x [128p, 256c].
                        psumT = psum_t_pool.tile([P, GPB * C], bf16, tag="pt",
                                                 padded_shape=[P, 1024])
                        tps = []
                        for g in range(GPB):
                            px = (bi * GPB + g) * P
                            t1 = nc.tensor.transpose(
                                psumT[:, g * C:g * C + P],
                                lo[:, px:px + P], identity)
                            t2 = nc.tensor.transpose(
                                psumT[:, g * C + P:(g + 1) * C],
                                hi[:, px:px + P], identity)
                            tps.extend((t1, t2))
                        for tp in tps:
                            order_after_mm(tp)
                        xT = work_pool.tile([P, GPB * C], bf16, tag="xT",
                                              bufs=6)
                        nc.vector.tensor_copy(xT[:], psumT[:])

                        ss = work_pool.tile([P, GPB], f32, tag="ss", bufs=32)
                        if batch % 2 == 0:
                            # Even batches: per-group ACT Square+accum.
                            for g in range(GPB):
                                sq = work_pool.tile([P, C], bf16, tag="sq",
                                                    bufs=8)
                                nc.scalar.activation(
                                    sq[:], xT[:, g * C:(g + 1) * C],
                                    mybir.ActivationFunctionType.Square,
                                    accum_out=ss[:, g:g + 1])
                        else:
                            # Odd batches: one batched ACT square + DVE reduce.
                            sq4 = work_pool.tile([P, GPB * C], bf16, tag="sq4",
                                                 bufs=8)
                            nc.scalar.activation(
                                sq4[:], xT[:],
                                mybir.ActivationFunctionType.Square)
                            nc.vector.tensor_reduce(
                                out=ss[:],
                                in_=sq4[:].rearrange("p (g c) -> p g c", g=GPB),
                                axis=mybir.AxisListType.X,
                                op=mybir.AluOpType.add)
                        nrm = work_pool.tile([P, GPB], f32, tag="nrm", bufs=32)
                        nc.scalar.sqrt(nrm[:], ss[:])
                        r = work_pool.tile([P, GPB], f32, tag="r", bufs=32)
                        nc.vector.reciprocal(r[:], nrm[:])

                        for g in range(GPB):
                            G = batch * GPB + g
                            w = w_pool.tile([P, N_CLASS], bf16, tag="w")
                            nc.vector.tensor_scalar(
                                out=w[:], in0=iota19,
                                scalar1=seg_sb[:, G:G + 1],
                                scalar2=r[:, g:g + 1],
                                op0=mybir.AluOpType.is_equal,
                                op1=mybir.AluOpType.mult)
                            for h in (0, 1):
                                mm = nc.tensor.matmul(
                                    accs[(t, h)][:],
                                    lhsT=xT[:, g * C + h * P:g * C + (h + 1) * P],
                                    rhs=w[:],
                                    start=(G == 0), stop=(G == NG - 1))
                                mm_all.append(mm)
                cbase += cpix // (P * GPB)

            out_sb = work_pool.tile([P, 4 * N_CLASS], f32, tag="out_sb")
            for j, (t, h) in enumerate(((("s", 0)), ("s", 1), ("c", 0), ("c", 1))):
                nc.vector.tensor_copy(
                    out_sb[:, j * N_CLASS:(j + 1) * N_CLASS], accs[(t, h)][:])
            for j, (ti, h) in enumerate(((0, 0), (0, 1), (1, 0), (1, 1))):
                # HWDGE lanes are otherwise unused -> each DMA carries only
                # its DVE wait.
                nc.sync.dma_start(
                    out=out_dram[ti, h],
                    in_=out_sb[:, j * N_CLASS:(j + 1) * N_CLASS])

    return nc


def aux_array():
    ident = np.eye(P, dtype=np.float32)
    iota = np.tile(np.arange(N_CLASS, dtype=np.float32), (P, 1))
    return np.ascontiguousarray(np.concatenate([ident, iota], axis=1))


def shard_inputs(f_source, f_convert, seg):
    """Split by (batch, h-half) into 8 per-core input maps."""
    in_maps = []
    hh = H // 2
    aux = aux_array()
    for core in range(N_CORES):
        b, half = divmod(core, 2)
        h0 = half * hh
        in_maps.append({
            "f_source": np.ascontiguousarray(
                f_source[b, :, h0:h0 + hh, :]).reshape(C, NPIX),
            "f_convert": np.ascontiguousarray(
                f_convert[b, :, h0:h0 + hh, :]).reshape(C, NPIX),
            "seg": np.ascontiguousarray(seg[b, h0:h0 + hh, :]).reshape(NPIX),
            "aux": aux,
        })
    return in_maps


def unpack_partial(p):
    """[2, 2, 128, 19] per-core partial -> (S, C) each [19, 256]."""
    s = np.concatenate([p[0, 0], p[0, 1]], axis=0).T
    c = np.concatenate([p[1, 0], p[1, 1]], axis=0).T
    return s, c


def epilogue(S, Csum):
    """Tiny triplet-loss tail on [19,256] class sums (float64 host math)."""
    n = float(B * H * W)
    cs = S.astype(np.float64) / n
    cc = Csum.astype(np.float64) / n
    cs = cs / np.maximum(np.linalg.norm(cs, axis=1, keepdims=True), EPS_NORM)
    cc = cc / np.maximum(np.linalg.norm(cc, axis=1, keepdims=True), EPS_NORM)
    D = np.linalg.norm(cs[:, None, :] - cc[None, :, :] + EPS_TRIP, axis=2)
    d_ap = np.diag(D)
    terms = np.maximum(d_ap[:, None] - D + MARGIN, 0.0)
    mask = 1.0 - np.eye(N_CLASS)
    loss = (terms * mask).sum() / (N_CLASS * (N_CLASS - 1))
    return np.float32(loss)


def kernel(f_source, f_convert, seg):
    if "nc" not in _NC_CACHE:
        _NC_CACHE["nc"] = build_nc()
    nc = _NC_CACHE["nc"]
    in_maps = shard_inputs(f_source, f_convert, seg)
    res = run_bass_kernel_spmd(nc, in_maps, core_ids=list(range(N_CORES)))
    S = np.zeros((N_CLASS, C), dtype=np.float64)
    Csum = np.zeros((N_CLASS, C), dtype=np.float64)
    for r in res.results:
        s, c = unpack_partial(r["out"].astype(np.float64))
        S += s
        Csum += c
    return epilogue(S, Csum)


if __name__ == "__main__":
    rng = np.random.default_rng(0)
    fs = rng.standard_normal((B, C, H, W), dtype=np.float32)
    fc = rng.standard_normal((B, C, H, W), dtype=np.float32)
    sg = rng.integers(0, N_CLASS, size=(B, H, W), dtype=np.int32)
    print(kernel(fs, fc, sg))



# revision 5
# speedup vs baseline: 1.4190x; 1.4190x over previous
"""Class-align loss (segment_reduce) Trainium2 kernel, v2.

Full inputs: f_source [4,256,128,128] f32, f_convert [4,256,128,128] f32,
seg [4,128,128] int32 (values in [0,19)). Output: scalar f32 triplet loss.

Strategy (data-parallel over batch*h-half, 8 shards; DMA-roofline bound):
  - Each core processes a [256, 8192] shard of each feature tensor.
    Staging DMAs cast fp32 -> bf16 in flight (SWDGE); per-core HBM read
    is 16.8 MB -> ~47 us floor at 358 GB/s.
  - Pixels are processed in batches of 8 groups x 128 pixels (one
    2-bank PSUM tile): PE transposes sixteen [128c,128p] bf16 blocks
    into psumT [128p, 8*256c]; one DVE copy moves it to SBUF.
  - Per-pixel norms: squares are computed on the NATURAL-layout staging
    tiles (lo half on ACT, hi half alternating ACT/DVE), then tiny
    ones-column PE matmuls reduce over channels directly into a
    partition-oriented PSUM tile ss[128p, 8] (2 matmuls per group,
    lo
    +hi accumulate).  This avoids the slow DVE tensor_reduce (1x mode)
    and per-group ACT accumulate ops entirely.
  - w generation is one DVE op per batch: host-precomputed one-hot
    (bf16, in the aux input) times a stride-0 broadcast AP of
    r = 1/||x||: w8[p, g*19+k] = onehot[p,...] * r[p, g].
  - Class sums accumulate with the data-stationary orientation
    (lhsT = xT chunk [128p,128c-half], rhs = w [128p,19]) into four
    [128, 19] f32 PSUM regions (s/c x lo/hi halves), 64 groups each.
  - The acc matmuls run one batch behind the transposes (software
    pipeline) so PE never stalls on the sqrt -> recip -> w chain.
  - Each core writes its [128, 4*19] partial sums; the host sums the 8
    partials and computes the tiny 19-class triplet-loss epilogue in
    float64.

The walrus build encodes at most ONE sync wait per instruction; the
emission order below is arranged so every instruction needs at most one
(absorber transposes take the staging-DMA waits on PE; the vector clock
subsumes all WAR hazards given the buffer depths used).
"""

import sys

import numpy as np

if "/opt/trn_rl_repo" not in sys.path:
    sys.path.insert(0, "/opt/trn_rl_repo")

import ml_dtypes

import concourse.bass as bass
import concourse.mybir as mybir
import concourse.tile as tile
from concourse.bass_utils import run_bass_kernel_spmd
from concourse.tile import add_dep_helper
from concourse.vector_clock import ScopedClock


def _split_drain_and_barrier(self, tick_clock, wait_clock):
    """Tile's kernel-tail drain carries one wait per semaphore the kernel
    ever used; split the excess onto dedicated sequencer NOPs (the 1-wait
    walrus encoding limit)."""
    nc = self.nc
    drain_inst = nc.sync.drain()
    wait_clock.add_sem_waits(
        drain_inst.ins, ScopedClock({None: tick_clock.global_clock})
    )
    si = drain_inst.ins.sync_info
    if si is not None and len(si.on_wait) > 1:
        waits = list(si.on_wait)
        upds = list(si.on_update)
        drain_inst.ins.sync_info = mybir.SyncInfo(
            on_wait=waits[:1], on_update=upds)
        for k in range(1, len(waits)):
            nop = nc.sync.nop(nofuse=True, hint=f"drain_wait_{k}")
            nop.ins.sync_info = mybir.SyncInfo(
                on_wait=[waits[k]], on_update=[])
    nc.all_engine_barrier()
    assert self.sems is not None
    popped = nc._tile_sem_poison_stack.pop()
    assert popped is self._sem_poison
    nc.clear_and_free_semaphores(list(self.sems.allocated().values()))
    nc.all_engine_barrier()


tile.TileContext._drain_and_barrier = _split_drain_and_barrier


def _split_excess_waits(nc):
    """Walrus encodes at most ONE sync wait per instruction; move any
    excess waits onto dedicated same-engine sequencer NOPs spliced in
    immediately before the offending instruction."""
    n = 0
    for f in nc.m.functions:
        for bb in f.blocks:
            out = []
            changed = False
            for ins in bb.instructions:
                si = ins.sync_info
                if si is not None and len(si.on_wait) > 1:
                    waits = list(si.on_wait)
                    for w in waits[:-1]:
                        nop = mybir.InstNoOp(name=f"I-waitsplit-{n}")
                        n += 1
                        nop.engine = ins.engine
                        nop.bass_nofuse = True
                        nop.sync_info = mybir.SyncInfo(on_wait=[w],
                                                       on_update=[])
                        out.append(nop)
                    ins.sync_info = mybir.SyncInfo(
                        on_wait=[waits[-1]], on_update=list(si.on_update))
                    changed = True
                out.append(ins)
            if changed:
                bb.instructions = out

# Problem constants (hardcoded; kernel.py must be self-contained).
B, C, H, W = 4, 256, 128, 128
N_CLASS = 19
N_CORES = 8
EPS_NORM = 1e-12
EPS_TRIP = 1e-6
MARGIN = 0.2

P = 128                      # SBUF partitions / pixel-group size
NPIX = B * H * W // N_CORES  # 8192 pixels per core
NG = NPIX // P               # 64 pixel groups per core
GPB = 8                      # pixel groups per batch (2 PSUM banks)
BPIX = GPB * P               # 1024 pixels per batch
NB = NG // GPB               # 8 batches per tensor
KW = GPB * N_CLASS           # 152 w columns per batch

# aux layout (bf16): identity [0:128) | ones col [128:129) | onehot
AUX_OH = P + 1
AUX_COLS = AUX_OH + NG * N_CLASS   # 129 + 1216 = 1345

# chunk pixel spans per (tensor, half): progressive then small tail
CHUNKS = [(0, 1024), (1024, 2048), (3072, 4096), (7168, 1024)]

_NC_CACHE = {}


def build_nc():
    f32 = mybir.dt.float32
    bf16 = mybir.dt.bfloat16
    Square = mybir.ActivationFunctionType.Square
    nc = bass.Bass()

    fs_dram = nc.declare_dram_parameter("f_source", [C, NPIX], f32,
                                        isOutput=False)
    aux_dram = nc.declare_dram_parameter("aux", [P, AUX_COLS], bf16,
                                         isOutput=False)
    fc_dram = nc.declare_dram_parameter("f_convert", [C, NPIX], f32,
                                        isOutput=False)
    out_dram = nc.declare_dram_parameter("out", [P, 4 * N_CLASS], f32,
                                         isOutput=True)
    drams = {"s": fs_dram, "c": fc_dram}

    with tile.TileContext(nc) as tc:
        with (
            tc.tile_pool(name="const", bufs=1) as const_pool,
            tc.tile_pool(name="stage", bufs=1) as stage_pool,
            tc.tile_pool(name="work", bufs=4) as work_pool,
            tc.tile_pool(name="psum_t", bufs=2, space="PSUM") as psum_t_pool,
            tc.tile_pool(name="psum_ss", bufs=2, space="PSUM") as psum_ss_pool,
            tc.tile_pool(name="psum_acc", bufs=1, space="PSUM") as psum_acc_pool,
            tc.tile_pool(name="psum_abs", bufs=1, space="PSUM") as psum_abs_pool,
        ):
            # Constants arrive via one HWDGE DMA (no Q7 involvement).
            aux_sb = const_pool.tile([P, AUX_COLS], bf16, tag="aux")
            nc.sync.dma_start(out=aux_sb[:], in_=aux_dram[:])
            identity = aux_sb[:, 0:P]
            ones_col = aux_sb[:, P:P + 1]

            # Transposed class-sum accumulators: 4 x [128 c-half, 19]
            # regions (s-lo, s-hi, c-lo, c-hi) in one PSUM bank.
            accs = psum_acc_pool.tile([P, 4 * N_CLASS], f32, tag="acc",
                                      name="accs", padded_shape=[P, 512])

            # Dedicated bank for DMA-wait absorber transposes (never read).
            absorb = psum_abs_pool.tile([P, 2 * P], bf16, tag="absorb",
                                        name="absorb", padded_shape=[P, 1024])

            # Warm-up transpose: takes the aux-DMA wait on PE so the first
            # real absorber carries only its staging-DMA wait.
            nc.tensor.transpose(absorb[:, 0:P], identity, identity)

            mm_all = []

            def order_after_mm(inst, back):
                if len(mm_all) >= back:
                    add_dep_helper(inst.ins, mm_all[-back].ins, sync=False,
                                   reason="pace Q7 descriptor generation")

            # ---- build the global batch schedule -------------------------
            # Each cycle processes one (tensor, chunk, batch-in-chunk).
            sched = []   # (t, ci, pix0, bi)
            for ci, (pix0, cpix) in enumerate(CHUNKS):
                for t in ("s", "c"):
                    for bi in range(cpix // BPIX):
                        sched.append((t, ci, pix0, bi))

            # staging tiles, dedicated per (chunk, tensor, half)
            stage = {}
            for ci, (pix0, cpix) in enumerate(CHUNKS):
                for t in ("s", "c"):
                    stage[(ci, t, "lo")] = stage_pool.tile(
                        [P, cpix], bf16, tag=f"{t}_lo_{ci}",
                        name=f"{t}_lo_{ci}")
                    stage[(ci, t, "hi")] = stage_pool.tile(
                        [P, cpix], bf16, tag=f"{t}_hi_{ci}",
                        name=f"{t}_hi_{ci}")

            group_cnt = {"s": 0, "c": 0}   # global group index per tensor
            tj = {"s": 0, "c": 1}
            issued_chunks = set()
            pend = None      # state of batch k-1 awaiting sqrt/recip/w/acc

            def emit_tail_of(pv):
                """sqrt(k-1), recip(k-1), w8(k-1) for the pending batch."""
                (t, g0, ss, xT) = pv
                nrm = work_pool.tile([P, GPB], f32, tag="nrm", bufs=4)
                nc.scalar.sqrt(nrm[:], ss[:])
                r = work_pool.tile([P, GPB], f32, tag="r", bufs=2)
                nc.vector.reciprocal(r[:], nrm[:])
                w8 = work_pool.tile([P, KW], bf16, tag="w8", bufs=2)
                oh = aux_sb[:, AUX_OH + g0 * N_CLASS:
                            AUX_OH + (g0 + GPB) * N_CLASS]
                nc.vector.tensor_tensor(
                    out=w8[:].rearrange("p (g k) -> p g k", g=GPB),
                    in0=oh.rearrange("p (g k) -> p g k", g=GPB),
                    in1=r[:, 0:GPB].broadcast_to([P, GPB, N_CLASS]),
                    op=mybir.AluOpType.mult)
                return (t, g0, xT, w8)

            def emit_acc_of(av):
                """acc matmuls for the completed batch (one batch behind)."""
                (t, g0, xT, w8) = av
                for g in range(GPB):
                    G = g0 + g
                    for h in (0, 1):
                        mm = nc.tensor.matmul(
                            accs[:, (tj[t] * 2 + h) * N_CLASS:
                                 (tj[t] * 2 + h + 1) * N_CLASS],
                            lhsT=xT[:, g * C + h * P:g * C + (h + 1) * P],
                            rhs=w8[:, g * N_CLASS:(g + 1) * N_CLASS],
                            start=(G == 0), stop=(G == NG - 1))
                        mm_all.append(mm)

            for k, (t, ci, pix0, bi) in enumerate(sched):
                lo = stage[(ci, t, "lo")]
                hi = stage[(ci, t, "hi")]
                cpix = CHUNKS[ci][1]

                if (ci, t) not in issued_chunks:
                    issued_chunks.add((ci, t))
                    # Staging DMAs (SWDGE casts fp32 -> bf16 in flight).
                    d1 = nc.gpsimd.dma_start(
                        out=lo[:], in_=drams[t][0:P, pix0:pix0 + cpix])
                    d2 = nc.gpsimd.dma_start(
                        out=hi[:], in_=drams[t][P:C, pix0:pix0 + cpix])
                    if ci >= 1:
                        order_after_mm(d1, back=48)
                        order_after_mm(d2, back=48)
                    # Absorbers take the DMA waits on PE so real transposes
                    # carry only their psumT WAR wait.
                    ab1 = nc.tensor.transpose(absorb[:, 0:P], lo[:, 0:P],
                                              identity)
                    ab2 = nc.tensor.transpose(absorb[:, P:2 * P], hi[:, 0:P],
                                              identity)
                    order_after_mm(ab1, back=24)
                    order_after_mm(ab2, back=24)

                g0 = group_cnt[t]
                group_cnt[t] += GPB
                b0 = bi * BPIX   # batch pixel offset within the chunk

                # --- PE: 16 transposes into psumT [128p, 8*256c] ----------
                psumT = psum_t_pool.tile([P, GPB * C], bf16, tag="pt",
                                         padded_shape=[P, GPB * C])
                for g in range(GPB):
                    px = b0 + g * P
                    t1 = nc.tensor.transpose(
                        psumT[:, g * C:g * C + P], lo[:, px:px + P], identity)
                    t2 = nc.tensor.transpose(
                        psumT[:, g * C + P:(g + 1) * C], hi[:, px:px + P],
                        identity)
                    mm_all.extend((t1, t2))

                # --- ACT: sqrt of k-1, then this batch's squares ----------
                pend_tail = emit_tail_of(pend) if pend is not None else None

                sq_lo = work_pool.tile([P, BPIX], bf16, tag="sq_lo", bufs=2)
                nc.scalar.activation(sq_lo[:], lo[:, b0:b0 + BPIX], Square)
                sq_hi = work_pool.tile([P, BPIX], bf16, tag="sq_hi", bufs=2)
                if k % 2 == 0:
                    nc.scalar.activation(sq_hi[:], hi[:, b0:b0 + BPIX],
                                         Square)
                else:
                    nc.vector.tensor_tensor(
                        out=sq_hi[:], in0=hi[:, b0:b0 + BPIX],
                        in1=hi[:, b0:b0 + BPIX], op=mybir.AluOpType.mult)

                # --- DVE: psumT -> SBUF copy ------------------------------
                xT = work_pool.tile([P, GPB * C], bf16, tag="xT", bufs=3)
                nc.vector.tensor_copy(xT[:], psumT[:])

                # --- PE: ones-column matmuls reduce squares to ss ---------
                ss = psum_ss_pool.tile([P, GPB], f32, tag="ss",
                                       padded_shape=[P, 512])
                for g in range(GPB):
                    mm = nc.tensor.matmul(
                        ss[:, g:g + 1], lhsT=sq_lo[:, g * P:(g + 1) * P],
                        rhs=ones_col, start=True, stop=False)
                    mm_all.append(mm)
                for g in range(GPB):
                    mm = nc.tensor.matmul(
                        ss[:, g:g + 1], lhsT=sq_hi[:, g * P:(g + 1) * P],
                        rhs=ones_col, start=False, stop=True)
                    mm_all.append(mm)

                # --- PE: acc matmuls of batch k-1 (software pipeline) -----
                if pend_tail is not None:
                    emit_acc_of(pend_tail)
                pend = (t, g0, ss, xT)

            # flush the last batch
            emit_acc_of(emit_tail_of(pend))

            out_sb = work_pool.tile([P, 4 * N_CLASS], f32, tag="out_sb")
            nc.vector.tensor_copy(out_sb[:], accs[:])
            nc.sync.dma_start(out=out_dram[:], in_=out_sb[:])

    _split_excess_waits(nc)
    return nc


def aux_array(seg_flat):
    """Per-core aux input: identity | ones | one-hot(seg), all bf16."""
    ident = np.eye(P, dtype=np.float32)
    ones = np.ones((P, 1), dtype=np.float32)
    segg = seg_flat.reshape(NG, P)                     # [group, p]
    oh = (segg[:, :, None] == np.arange(N_CLASS)[None, None, :])
    oh = oh.astype(np.float32).transpose(1, 0, 2).reshape(P, NG * N_CLASS)
    aux = np.concatenate([ident, ones, oh], axis=1)
    return np.ascontiguousarray(aux.astype(ml_dtypes.bfloat16))


def shard_inputs(f_source, f_convert, seg):
    """Split by (batch, h-half) into 8 per-core input maps."""
    in_maps = []
    hh = H // 2
    for core in range(N_CORES):
        b, half = divmod(core, 2)
        h0 = half * hh
        seg_flat = np.ascontiguousarray(seg[b, h0:h0 + hh, :]).reshape(NPIX)
        in_maps.append({
            "f_source": np.ascontiguousarray(
                f_source[b, :, h0:h0 + hh, :]).reshape(C, NPIX),
            "f_convert": np.ascontiguousarray(
                f_convert[b, :, h0:h0 + hh, :]).reshape(C, NPIX),
            "aux": aux_array(seg_flat),
        })
    return in_maps


def unpack_partial(p):
    """[128, 4*19] per-core partial -> (S, C) each [19, 256]."""
    blocks = [p[:, j * N_CLASS:(j + 1) * N_CLASS] for j in range(4)]
    s = np.concatenate([blocks[0], blocks[1]], axis=0).T
    c = np.concatenate([blocks[2], blocks[3]], axis=0).T
    return s, c


def epilogue(S, Csum):
    """Tiny triplet-loss tail on [19,256] class sums (float64 host math)."""
    n = float(B * H * W)
    cs = S.astype(np.float64) / n
    cc = Csum.astype(np.float64) / n
    cs = cs / np.maximum(np.linalg.norm(cs, axis=1, keepdims=True), EPS_NORM)
    cc = cc / np.maximum(np.linalg.norm(cc, axis=1, keepdims=True), EPS_NORM)
    D = np.linalg.norm(cs[:, None, :] - cc[None, :, :] + EPS_TRIP, axis=2)
    d_ap = np.diag(D)
    terms = np.maximum(d_ap[:, None] - D + MARGIN, 0.0)
    mask = 1.0 - np.eye(N_CLASS)
    loss = (terms * mask).sum() / (N_CLASS * (N_CLASS - 1))
    return np.float32(loss)


def kernel(f_source, f_convert, seg):
    if "nc" not in _NC_CACHE:
        _NC_CACHE["nc"] = build_nc()
    nc = _NC_CACHE["nc"]
    in_maps = shard_inputs(f_source, f_convert, seg)
    res = run_bass_kernel_spmd(nc, in_maps, core_ids=list(range(N_CORES)))
    S = np.zeros((N_CLASS, C), dtype=np.float64)
    Csum = np.zeros((N_CLASS, C), dtype=np.float64)
    for r in res.results:
        s, c = unpack_partial(r["out"].astype(np.float64))
        S += s
        Csum += c
    return epilogue(S, Csum)


if __name__ == "__main__":
    rng = np.random.default_rng(0)
    fs = rng.standard_normal((B, C, H, W), dtype=np.float32)
    fc = rng.standard_normal((B, C, H, W), dtype=np.float32)
    sg = rng.integers(0, N_CLASS, size=(B, H, W), dtype=np.int32)
    print(kernel(fs, fc, sg))


# revision 9
# speedup vs baseline: 1.4217x; 1.0019x over previous
"""Class-align loss (segment_reduce) Trainium2 kernel, v2.

Full inputs: f_source [4,256,128,128] f32, f_convert [4,256,128,128] f32,
seg [4,128,128] int32 (values in [0,19)). Output: scalar f32 triplet loss.

Strategy (data-parallel over batch*h-half, 8 shards; DMA-roofline bound):
  - Each core processes a [256, 8192] shard of each feature tensor.
    Staging DMAs cast fp32 -> bf16 in flight (SWDGE); per-core HBM read
    is 16.8 MB -> ~47 us floor at 358 GB/s.
  - Pixels are processed in batches of 8 groups x 128 pixels (one
    2-bank PSUM tile): PE transposes sixteen [128c,128p] bf16 blocks
    into psumT [128p, 8*256c]; one DVE copy moves it to SBUF.
  - Per-pixel norms: squares are computed on the NATURAL-layout staging
    tiles (lo half on ACT, hi half alternating ACT/DVE), then tiny
    ones-column PE matmuls reduce over channels directly into a
    partition-oriented PSUM tile ss[128p, 8] (2 matmuls per group,
    lo
    +hi accumulate).  This avoids the slow DVE tensor_reduce (1x mode)
    and per-group ACT accumulate ops entirely.
  - w generation is one DVE op per batch: host-precomputed one-hot
    (bf16, in the aux input) times a stride-0 broadcast AP of
    r = 1/||x||: w8[p, g*19+k] = onehot[p,...] * r[p, g].
  - Class sums accumulate with the data-stationary orientation
    (lhsT = xT chunk [128p,128c-half], rhs = w [128p,19]) into four
    [128, 19] f32 PSUM regions (s/c x lo/hi halves), 64 groups each.
  - The acc matmuls run one batch behind the transposes (software
    pipeline) so PE never stalls on the sqrt -> recip -> w chain.
  - Each core writes its [128, 4*19] partial sums; the host sums the 8
    partials and computes the tiny 19-class triplet-loss epilogue in
    float64.

The walrus build encodes at most ONE sync wait per instruction; the
emission order below is arranged so every instruction needs at most one
(absorber transposes take the staging-DMA waits on PE; the vector clock
subsumes all WAR hazards given the buffer depths used).
"""

import sys

import numpy as np

if "/opt/trn_rl_repo" not in sys.path:
    sys.path.insert(0, "/opt/trn_rl_repo")

import ml_dtypes

import concourse.bass as bass
import concourse.mybir as mybir
import concourse.tile as tile
from concourse.bass_utils import run_bass_kernel_spmd
from concourse.tile import add_dep_helper
from concourse.vector_clock import ScopedClock


def _split_drain_and_barrier(self, tick_clock, wait_clock):
    """Tile's kernel-tail drain carries one wait per semaphore the kernel
    ever used; split the excess onto dedicated sequencer NOPs (the 1-wait
    walrus encoding limit)."""
    nc = self.nc
    drain_inst = nc.sync.drain()
    wait_clock.add_sem_waits(
        drain_inst.ins, ScopedClock({None: tick_clock.global_clock})
    )
    si = drain_inst.ins.sync_info
    if si is not None and len(si.on_wait) > 1:
        waits = list(si.on_wait)
        upds = list(si.on_update)
        drain_inst.ins.sync_info = mybir.SyncInfo(
            on_wait=waits[:1], on_update=upds)
        for k in range(1, len(waits)):
            nop = nc.sync.nop(nofuse=True, hint=f"drain_wait_{k}")
            nop.ins.sync_info = mybir.SyncInfo(
                on_wait=[waits[k]], on_update=[])
    nc.all_engine_barrier()
    assert self.sems is not None
    popped = nc._tile_sem_poison_stack.pop()
    assert popped is self._sem_poison
    nc.clear_and_free_semaphores(list(self.sems.allocated().values()))
    nc.all_engine_barrier()


tile.TileContext._drain_and_barrier = _split_drain_and_barrier


def _split_excess_waits(nc):
    """Walrus encodes at most ONE sync wait per instruction; move any
    excess waits onto dedicated same-engine sequencer NOPs spliced in
    immediately before the offending instruction."""
    n = 0
    for f in nc.m.functions:
        for bb in f.blocks:
            out = []
            changed = False
            for ins in bb.instructions:
                si = ins.sync_info
                if si is not None and len(si.on_wait) > 1:
                    waits = list(si.on_wait)
                    for w in waits[:-1]:
                        nop = mybir.InstNoOp(name=f"I-waitsplit-{n}")
                        n += 1
                        nop.engine = ins.engine
                        nop.bass_nofuse = True
                        nop.sync_info = mybir.SyncInfo(on_wait=[w],
                                                       on_update=[])
                        out.append(nop)
                    ins.sync_info = mybir.SyncInfo(
                        on_wait=[waits[-1]], on_update=list(si.on_update))
                    changed = True
                out.append(ins)
            if changed:
                bb.instructions = out

# Problem constants (hardcoded; kernel.py must be self-contained).
B, C, H, W = 4, 256, 128, 128
N_CLASS = 19
N_CORES = 8
EPS_NORM = 1e-12
EPS_TRIP = 1e-6
MARGIN = 0.2

P = 128                      # SBUF partitions / pixel-group size
NPIX = B * H * W // N_CORES  # 8192 pixels per core
NG = NPIX // P               # 64 pixel groups per core
GPB = 8                      # pixel groups per batch (2 PSUM banks)
BPIX = GPB * P               # 1024 pixels per batch
NB = NG // GPB               # 8 batches per tensor
KW = GPB * N_CLASS           # 152 w columns per batch

# aux layout (bf16): identity [0:128) | ones col [128:129) | onehot
AUX_OH = P + 1
AUX_COLS = AUX_OH + NG * N_CLASS   # 129 + 1216 = 1345

# chunk pixel spans per (tensor, half): progressive then small tail
CHUNKS = [(0, 1024), (1024, 2048), (3072, 4096), (7168, 1024)]

_NC_CACHE = {}


def build_nc():
    f32 = mybir.dt.float32
    bf16 = mybir.dt.bfloat16
    Square = mybir.ActivationFunctionType.Square
    nc = bass.Bass()

    fs_dram = nc.declare_dram_parameter("f_source", [C, NPIX], f32,
                                        isOutput=False)
    aux_dram = nc.declare_dram_parameter("aux", [P, AUX_COLS], bf16,
                                         isOutput=False)
    fc_dram = nc.declare_dram_parameter("f_convert", [C, NPIX], f32,
                                        isOutput=False)
    out_dram = nc.declare_dram_parameter("out", [P, 4 * N_CLASS], f32,
                                         isOutput=True)
    drams = {"s": fs_dram, "c": fc_dram}

    with tile.TileContext(nc) as tc:
        with (
            tc.tile_pool(name="const", bufs=1) as const_pool,
            tc.tile_pool(name="stage", bufs=1) as stage_pool,
            tc.tile_pool(name="work", bufs=4) as work_pool,
            tc.tile_pool(name="psum_t", bufs=3, space="PSUM") as psum_t_pool,
            tc.tile_pool(name="psum_ss", bufs=1, space="PSUM") as psum_ss_pool,
            tc.tile_pool(name="psum_acc", bufs=1, space="PSUM") as psum_acc_pool,
        ):
            # Constants arrive via one HWDGE DMA (no Q7 involvement).
            aux_sb = const_pool.tile([P, AUX_COLS], bf16, tag="aux")
            nc.sync.dma_start(out=aux_sb[:], in_=aux_dram[:])
            identity = aux_sb[:, 0:P]
            ones_col = aux_sb[:, P:P + 1]

            # Transposed class-sum accumulators: 4 x [128 c-half, 19]
            # regions (s-lo, s-hi, c-lo, c-hi) in one PSUM bank.
            accs = psum_acc_pool.tile([P, 4 * N_CLASS], f32, tag="acc",
                                      name="accs", padded_shape=[P, 512])

            mm_all = []

            def order_after_mm(inst, back):
                if len(mm_all) >= back:
                    add_dep_helper(inst.ins, mm_all[-back].ins, sync=False,
                                   reason="pace Q7 descriptor generation")

            # ---- build the global batch schedule -------------------------
            # Each cycle processes one (tensor, chunk, batch-in-chunk).
            sched = []   # (t, ci, pix0, bi)
            for ci, (pix0, cpix) in enumerate(CHUNKS):
                for t in ("s", "c"):
                    for bi in range(cpix // BPIX):
                        sched.append((t, ci, pix0, bi))
            NBT = len(sched)

            # staging tiles, dedicated per (chunk, tensor, half)
            stage = {}
            for ci, (pix0, cpix) in enumerate(CHUNKS):
                for t in ("s", "c"):
                    stage[(ci, t, "lo")] = stage_pool.tile(
                        [P, cpix], bf16, tag=f"{t}_lo_{ci}",
                        name=f"{t}_lo_{ci}")
                    stage[(ci, t, "hi")] = stage_pool.tile(
                        [P, cpix], bf16, tag=f"{t}_hi_{ci}",
                        name=f"{t}_hi_{ci}")

            # Warm-up transposes: take the aux-DMA wait on PE and spin the
            # HAM activity monitor up to 2.4 GHz during the DMA-wait ramp.
            warm = psum_t_pool.tile([P, GPB * C], bf16, tag="pt",
                                    name="warm", padded_shape=[P, GPB * C])
            for _ in range(14):
                nc.tensor.transpose(warm[:, 0:P], identity, identity)

            group_cnt = {"s": 0, "c": 0}   # global group index per tensor
            tj = {"s": 0, "c": 1}
            issued_chunks = set()
            # Per-batch state for the software pipeline (lag structure:
            # cycle k runs T(k) | O(k-1) | A(k-3) on PE).
            meta = {}    # k -> (t, g0, lo, hi, b0)
            sqlo = {}
            sqhi = {}
            sshandle = {}
            psumT = {}
            xT = {}
            w8 = {}

            def emit_T(k, g, h):
                m = meta[k]
                src = m[2] if h == 0 else m[3]
                px = m[4] + g * P
                mm = nc.tensor.transpose(
                    psumT[k][:, g * C + h * P:g * C + (h + 1) * P],
                    src[:, px:px + P], identity)
                mm_all.append(mm)

            def emit_O(k, g, h):
                sq = sqlo[k] if h == 0 else sqhi[k]
                mm = nc.tensor.matmul(
                    sshandle[k][:, g:g + 1], lhsT=sq[:, g * P:(g + 1) * P],
                    rhs=ones_col, start=(h == 0), stop=(h == 1))
                mm_all.append(mm)

            def emit_A(k, g, h):
                t, g0 = meta[k][0], meta[k][1]
                G = g0 + g
                mm = nc.tensor.matmul(
                    accs[:, (tj[t] * 2 + h) * N_CLASS:
                         (tj[t] * 2 + h + 1) * N_CLASS],
                    lhsT=xT[k][:, g * C + h * P:g * C + (h + 1) * P],
                    rhs=w8[k][:, g * N_CLASS:(g + 1) * N_CLASS],
                    start=(G == 0), stop=(G == NG - 1))
                mm_all.append(mm)

            for k in range(NBT + 3):
                hasT = k < NBT
                hasO = 0 <= k - 1 < NBT
                hasA = 0 <= k - 3 < NBT
                chunk_first = False

                if hasT:
                    t, ci, pix0, bi = sched[k]
                    lo = stage[(ci, t, "lo")]
                    hi = stage[(ci, t, "hi")]
                    cpix = CHUNKS[ci][1]
                    if (ci, t) not in issued_chunks:
                        issued_chunks.add((ci, t))
                        chunk_first = True
                        # SWDGE staging DMAs cast fp32 -> bf16 in flight.
                        d1 = nc.gpsimd.dma_start(
                            out=lo[:], in_=drams[t][0:P, pix0:pix0 + cpix])
                        d2 = nc.gpsimd.dma_start(
                            out=hi[:], in_=drams[t][P:C, pix0:pix0 + cpix])
                        if ci >= 1:
                            order_after_mm(d1, back=48)
                            order_after_mm(d2, back=48)
                    g0 = group_cnt[t]
                    group_cnt[t] += GPB
                    meta[k] = (t, g0, lo, hi, bi * BPIX)
                    psumT[k] = psum_t_pool.tile([P, GPB * C], bf16, tag="pt",
                                                name=f"psumT_{k}",
                                                padded_shape=[P, GPB * C])

                # --- PE: interleave O(k-1) | A(k-3) | T(k) ----------------
                # (T last in each sextet: the post-transpose pipeline bubble
                # lands on cheap O/A issues, not on the next transpose.)
                if chunk_first and (hasO or hasA):
                    # Chunk-first cycles: old work first so the PE queue has
                    # useful instructions while the chunk's DMA completes.
                    for g in range(GPB):
                        if hasO:
                            emit_O(k - 1, g, 0)
                            emit_O(k - 1, g, 1)
                        if hasA:
                            emit_A(k - 3, g, 0)
                            emit_A(k - 3, g, 1)
                    for g in range(GPB):
                        emit_T(k, g, 0)
                        emit_T(k, g, 1)
                else:
                    for g in range(GPB):
                        for h in (0, 1):
                            if hasO:
                                emit_O(k - 1, g, h)
                            if hasA:
                                emit_A(k - 3, g, h)
                            if hasT:
                                emit_T(k, g, h)

                # --- ACT: squares of batch k, then sqrt of k-1 ------------
                if hasT:
                    m = meta[k]
                    lo, hi, b0 = m[2], m[3], m[4]
                    sqlo[k] = work_pool.tile([P, BPIX], bf16, tag="sq_lo",
                                             name=f"sqlo_{k}", bufs=2)
                    nc.scalar.activation(sqlo[k][:], lo[:, b0:b0 + BPIX],
                                         Square)
                    sqhi[k] = work_pool.tile([P, BPIX], bf16, tag="sq_hi",
                                             name=f"sqhi_{k}", bufs=2)
                    if k % 2 == 0:
                        nc.scalar.activation(sqhi[k][:], hi[:, b0:b0 + BPIX],
                                             Square)
                if hasO:
                    nrm = work_pool.tile([P, GPB], f32, tag="nrm", bufs=4)
                    nc.scalar.sqrt(nrm[:], sshandle[k - 1][:])
                    meta[k - 1] += (nrm,)

                # --- DVE: recip/w8 of k-2, sq_hi(k), copy of k-1 ----------
                if 0 <= k - 2 < NBT:
                    nrm2 = meta[k - 2][5]
                    r = work_pool.tile([P, GPB], f32, tag="r", bufs=2)
                    nc.vector.reciprocal(r[:], nrm2[:])
                    w8[k - 2] = work_pool.tile([P, KW], bf16, tag="w8",
                                               name=f"w8_{k - 2}", bufs=3)
                    g0 = meta[k - 2][1]
                    oh = aux_sb[:, AUX_OH + g0 * N_CLASS:
                                AUX_OH + (g0 + GPB) * N_CLASS]
                    nc.vector.tensor_tensor(
                        out=w8[k - 2][:].rearrange("p (g k) -> p g k", g=GPB),
                        in0=oh.rearrange("p (g k) -> p g k", g=GPB),
                        in1=r[:, 0:GPB].broadcast_to([P, GPB, N_CLASS]),
                        op=mybir.AluOpType.mult)
                if hasT and k % 2 == 1:
                    m = meta[k]
                    nc.vector.tensor_tensor(
                        out=sqhi[k][:], in0=m[3][:, m[4]:m[4] + BPIX],
                        in1=m[3][:, m[4]:m[4] + BPIX], op=mybir.AluOpType.mult)
                if hasO:
                    xT[k - 1] = work_pool.tile([P, GPB * C], bf16, tag="xT",
                                               name=f"xT_{k - 1}", bufs=4)
                    nc.vector.tensor_copy(xT[k - 1][:], psumT[k - 1][:])

                # ss tile for batch k (written by O(k) next cycle).
                if hasT:
                    sshandle[k] = psum_ss_pool.tile([P, GPB], f32, tag="ss",
                                                    name=f"ss_{k}",
                                                    padded_shape=[P, 512])

            out_sb = work_pool.tile([P, 4 * N_CLASS], f32, tag="out_sb")
            nc.vector.tensor_copy(out_sb[:], accs[:])
            nc.sync.dma_start(out=out_dram[:], in_=out_sb[:])

    _split_excess_waits(nc)
    return nc


def aux_array(seg_flat):
    """Per-core aux input: identity | ones | one-hot(seg), all bf16."""
    ident = np.eye(P, dtype=np.float32)
    ones = np.ones((P, 1), dtype=np.float32)
    segg = seg_flat.reshape(NG, P)                     # [group, p]
    oh = (segg[:, :, None] == np.arange(N_CLASS)[None, None, :])
    oh = oh.astype(np.float32).transpose(1, 0, 2).reshape(P, NG * N_CLASS)
    aux = np.concatenate([ident, ones, oh], axis=1)
    return np.ascontiguousarray(aux.astype(ml_dtypes.bfloat16))


def shard_inputs(f_source, f_convert, seg):
    """Split by (batch, h-half) into 8 per-core input maps."""
    in_maps = []
    hh = H // 2
    for core in range(N_CORES):
        b, half = divmod(core, 2)
        h0 = half * hh
        seg_flat = np.ascontiguousarray(seg[b, h0:h0 + hh, :]).reshape(NPIX)
        in_maps.append({
            "f_source": np.ascontiguousarray(
                f_source[b, :, h0:h0 + hh, :]).reshape(C, NPIX),
            "f_convert": np.ascontiguousarray(
                f_convert[b, :, h0:h0 + hh, :]).reshape(C, NPIX),
            "aux": aux_array(seg_flat),
        })
    return in_maps


def unpack_partial(p):
    """[128, 4*19] per-core partial -> (S, C) each [19, 256]."""
    blocks = [p[:, j * N_CLASS:(j + 1) * N_CLASS] for j in range(4)]
    s = np.concatenate([blocks[0], blocks[1]], axis=0).T
    c = np.concatenate([blocks[2], blocks[3]], axis=0).T
    return s, c


def epilogue(S, Csum):
    """Tiny triplet-loss tail on [19,256] class sums (float64 host math)."""
    n = float(B * H * W)
    cs = S.astype(np.float64) / n
    cc = Csum.astype(np.float64) / n
    cs = cs / np.maximum(np.linalg.norm(cs, axis=1, keepdims=True), EPS_NORM)
    cc = cc / np.maximum(np.linalg.norm(cc, axis=1, keepdims=True), EPS_NORM)
    D = np.linalg.norm(cs[:, None, :] - cc[None, :, :] + EPS_TRIP, axis=2)
    d_ap = np.diag(D)
    terms = np.maximum(d_ap[:, None] - D + MARGIN, 0.0)
    mask = 1.0 - np.eye(N_CLASS)
    loss = (terms * mask).sum() / (N_CLASS * (N_CLASS - 1))
    return np.float32(loss)


def kernel(f_source, f_convert, seg):
    if "nc" not in _NC_CACHE:
        _NC_CACHE["nc"] = build_nc()
    nc = _NC_CACHE["nc"]
    in_maps = shard_inputs(f_source, f_convert, seg)
    res = run_bass_kernel_spmd(nc, in_maps, core_ids=list(range(N_CORES)))
    S = np.zeros((N_CLASS, C), dtype=np.float64)
    Csum = np.zeros((N_CLASS, C), dtype=np.float64)
    for r in res.results:
        s, c = unpack_partial(r["out"].astype(np.float64))
        S += s
        Csum += c
    return epilogue(S, Csum)


if __name__ == "__main__":
    rng = np.random.default_rng(0)
    fs = rng.standard_normal((B, C, H, W), dtype=np.float32)
    fc = rng.standard_normal((B, C, H, W), dtype=np.float32)
    sg = rng.integers(0, N_CLASS, size=(B, H, W), dtype=np.int32)
    print(kernel(fs, fc, sg))
